# revision 1
# baseline (speedup 1.0000x reference)
"""Trainium2 Bass kernel for nn_AttentionPageExample (paged KV-cache scatter).

The reference computes h = ones; for each layer: xk=2h, xv=4h,
slab[id*12+2l]=xk-blocks, slab[id*12+2l+1]=xv-blocks, h += h*xk*xv.
Since h starts uniform and all updates are elementwise-uniform, h stays a
single scalar per layer (1, 9, 5841, ... -> inf) independent of all inputs.
Hence the true work is pure memory movement:
  - every page whose index appears in attn_block_ids is overwritten with a
    fixed 3MB constant pattern (k/v fill values per layer),
  - every other page is copied through unchanged,
  - the h output is a constant (inf) everywhere.

Sharding: page-parallel across 8 cores (vLLM-style). Host routes each core
its copy-source pages; fills are synthesized on device from SBUF memsets.
Per-core device work: 4 fill pages (12MB write), 12 copy pages (36MB read +
36MB write, direct DRAM->DRAM DMA), 1MB h write.
"""

import numpy as np

import concourse.bass as bass
import concourse.mybir as mybir
from concourse.bass_utils import run_bass_kernel_spmd

N_CORES = 8
PAGES = 128
LAYERS = 6
CACHE_LINES = 2
STRIDE = 16
HEADS = 32
HEAD_DIM = 128
BS = 4
SEQ = 128
FEAT = HEADS * HEAD_DIM                      # 4096
SUB = STRIDE * HEADS * HEAD_DIM              # 65536 elems per sub-block
SUBS_PER_PAGE = LAYERS * CACHE_LINES         # 12
PAGE_ELEMS = SUBS_PER_PAGE * SUB             # 786432 elems (3MB)
H_ELEMS = BS * SEQ * FEAT                    # 2097152 elems (8MB)
H_PER_CORE = H_ELEMS // N_CORES              # 262144 elems (1MB)
H_COLS = H_PER_CORE // 128                   # 2048

SLAB_SHAPE = (PAGES, LAYERS, CACHE_LINES, STRIDE, HEADS, HEAD_DIM)


def _fill_values():
    """Replicate the reference's f32 scalar recurrence exactly."""
    h = np.float32(1.0)
    ks, vs = [], []
    with np.errstate(over="ignore", invalid="ignore"):
        for _ in range(LAYERS):
            xk = np.float32(2.0) * h
            xv = np.float32(4.0) * h
            ks.append(xk)
            vs.append(xv)
            h = np.float32(h + (h * xk) * xv)
    return ks, vs, h


_NC_CACHE = {}


def build_nc(F, C, reps=1):
    """Uniform SPMD program: F fill-pages + C copy-pages + h shard per core.

    reps > 1 repeats the DMA workload (for slope-based timing only).
    """
    key = (F, C, reps)
    if key in _NC_CACHE:
        return _NC_CACHE[key]
    ks, vs, h_fin = _fill_values()
    f32 = mybir.dt.float32
    nc = bass.Bass()
    src = nc.declare_dram_parameter("src_pages", [C, SUBS_PER_PAGE, 128, 512], f32, isOutput=False)
    outp = nc.declare_dram_parameter("out_pages", [F + C, SUBS_PER_PAGE, 128, 512], f32, isOutput=True)
    outh = nc.declare_dram_parameter("out_h", [128, H_COLS], f32, isOutput=True)

    with (
        nc.sbuf_tensor([128, SUBS_PER_PAGE * 512], f32) as ptile,
        nc.sbuf_tensor([128, H_COLS], f32) as htile,
        nc.semaphore("vsem") as vsem,
        nc.semaphore("s_sem") as s_sem,
        nc.semaphore("a_sem") as a_sem,
        nc.Block() as block,
    ):
        @block.vector
        def _(vector):
            for l in range(LAYERS):
                vector.memset(ptile[:, (2 * l) * 512:(2 * l + 1) * 512], float(ks[l]))
                vector.memset(ptile[:, (2 * l + 1) * 512:(2 * l + 2) * 512], float(vs[l]))
            vector.memset(htile[:, :], float(h_fin)).then_inc(vsem, 1)

        # Copy pages: direct DRAM->DRAM on the SP HWDGE ring.
        @block.sync
        def _(sync):
            n = 0
            for _ in range(reps):
                for j in range(C):
                    sync.dma_start(out=outp[F + j], in_=src[j]).then_inc(s_sem, 16)
                    n += 16
            sync.wait_ge(s_sem, n)

        # Fill pages + h: SBUF->DRAM on the ACT HWDGE ring.
        @block.scalar
        def _(scalar):
            scalar.wait_ge(vsem, 1)
            n = 0
            for _ in range(reps):
                for f in range(F):
                    scalar.dma_start(
                        out=outp[f].rearrange("k p c -> p k c"),
                        in_=ptile[:, :],
                    ).then_inc(a_sem, 16)
                    n += 16
                scalar.dma_start(out=outh[:, :], in_=htile[:, :]).then_inc(a_sem, 16)
                n += 16
            scalar.wait_ge(a_sem, n)

    _NC_CACHE[key] = nc
    return nc


def plan(attn_block_ids):
    """Host-side routing: which pages are constant-filled vs copied, and the
    per-core slot assignment (padded to uniform F/C counts)."""
    ids = np.asarray(attn_block_ids).reshape(-1).astype(np.int64)
    ids = ids[(ids >= 0) & (ids < PAGES)]
    fill_pages = np.unique(ids)
    mask = np.zeros(PAGES, dtype=bool)
    mask[fill_pages] = True
    copy_pages = np.nonzero(~mask)[0]
    nf, ncp = len(fill_pages), len(copy_pages)
    F = max(1, -(-nf // N_CORES))
    C = max(1, -(-ncp // N_CORES))
    fill_asn = np.full(F * N_CORES, -1, dtype=np.int64)
    fill_asn[:nf] = fill_pages
    copy_asn = np.full(C * N_CORES, -1, dtype=np.int64)
    copy_asn[:ncp] = copy_pages
    return F, C, fill_asn.reshape(N_CORES, F), copy_asn.reshape(N_CORES, C)


def make_in_maps(attn_page_slab, copy_asn):
    slab_pages = np.asarray(attn_page_slab).reshape(PAGES, SUBS_PER_PAGE, 128, 512)
    in_maps = []
    for c in range(N_CORES):
        pl = np.where(copy_asn[c] >= 0, copy_asn[c], 0)
        in_maps.append({"src_pages": np.ascontiguousarray(slab_pages[pl])})
    return in_maps


def assemble(results, F, fill_asn, copy_asn, slab_dtype):
    out_slab = np.empty((PAGES, SUBS_PER_PAGE, 128, 512), dtype=slab_dtype)
    for c in range(N_CORES):
        op = results[c]["out_pages"]
        for i, p in enumerate(fill_asn[c]):
            if p >= 0:
                out_slab[p] = op[i]
        for j, p in enumerate(copy_asn[c]):
            if p >= 0:
                out_slab[p] = op[F + j]
    h_rows = np.concatenate([results[c]["out_h"] for c in range(N_CORES)], axis=0)
    h = h_rows.reshape(BS, SEQ, FEAT)
    return h, out_slab.reshape(SLAB_SHAPE)


def kernel(seq_lens=None, attn_block_ids=None, attn_page_slab=None, **_):
    slab = np.asarray(attn_page_slab)
    F, C, fill_asn, copy_asn = plan(attn_block_ids)
    nc = build_nc(F, C)
    in_maps = make_in_maps(slab, copy_asn)
    res = run_bass_kernel_spmd(nc, in_maps, list(range(N_CORES)))
    return assemble(res.results, F, fill_asn, copy_asn, slab.dtype)


# revision 2
# speedup vs baseline: 6.0455x; 6.0455x over previous
"""Trainium2 Bass kernel for nn_AttentionPageExample (paged KV-cache scatter).

The reference computes h = ones; per layer: xk=2h, xv=4h, scatter xk/xv blocks
into the page slab at indices derived from attn_block_ids, h += h*xk*xv.
Since h starts uniform and every update is elementwise-uniform, h is a single
scalar per layer (1, 9, 5841, ... -> inf), independent of all inputs. Every
page whose index appears in attn_block_ids gets all 12 sub-blocks overwritten
with per-layer constants; all other pages pass through unchanged; the h output
is a uniform constant (inf).

Strategy (vLLM-style page-parallel, in-place):
  - Host routes the 128 pages across 8 cores, 16 slots each; each core's
    fill-target pages are placed in its first F slots.
  - The page buffer is declared input AND output with the output aliased to
    the input buffer (in-place page-cache update), so untouched pages move
    zero bytes on device.
  - Device per core: build the 3MB constant page pattern in SBUF (12 memsets)
    and scatter it into the F fill slots, sub-block-granular so DMA starts as
    soon as the first memset lands; h shard (1MB) is written from a small
    constant tile on the second DMA ring. ~13MB of HBM writes per core.
  - A host-side spot-check verifies the aliasing actually passed the copy
    pages through; if not, we rerun with an explicit copy kernel.
"""

import numpy as np

import concourse.bass as bass
import concourse.mybir as mybir
from concourse._compat import axon_active
from concourse.bass_utils import run_bass_kernel_spmd

N_CORES = 8
PAGES = 128
LAYERS = 6
CACHE_LINES = 2
STRIDE = 16
HEADS = 32
HEAD_DIM = 128
BS = 4
SEQ = 128
FEAT = HEADS * HEAD_DIM                      # 4096
SUB = STRIDE * HEADS * HEAD_DIM              # 65536 elems per sub-block
SUBS = LAYERS * CACHE_LINES                  # 12 sub-blocks per page
PAGE_ELEMS = SUBS * SUB                      # 786432 elems (3MB)
H_ELEMS = BS * SEQ * FEAT                    # 2097152 elems (8MB)
H_PER_CORE = H_ELEMS // N_CORES              # 262144 elems (1MB)

SLAB_SHAPE = (PAGES, LAYERS, CACHE_LINES, STRIDE, HEADS, HEAD_DIM)
F32 = mybir.dt.float32


def _fill_values():
    """The reference's f32 scalar recurrence, replicated exactly."""
    h = np.float32(1.0)
    ks, vs = [], []
    with np.errstate(over="ignore", invalid="ignore"):
        for _ in range(LAYERS):
            xk = np.float32(2.0) * h
            xv = np.float32(4.0) * h
            ks.append(xk)
            vs.append(xv)
            h = np.float32(h + (h * xk) * xv)
    return ks, vs, h


_NC_CACHE = {}


def build_inplace(F, C, reps=1):
    """In-place variant: io_pages is input + (aliased) output; only the first
    F slots are overwritten with the constant pattern. Sub-block-granular fill
    DMAs split across the SP and ACT HWDGE rings; h written from ACT ring."""
    key = ("inplace", F, C, reps)
    if key in _NC_CACHE:
        return _NC_CACHE[key]
    ks, vs, h_fin = _fill_values()
    G = F + C
    nc = bass.Bass()
    iop = nc.declare_dram_parameter("io_pages", [G, SUBS, 128, 512], F32, isOutput=False)
    nc.declare_dram_parameter("out_pages", [G, SUBS, 128, 512], F32, isOutput=True)
    outh = nc.declare_dram_parameter("out_h", [512, 512], F32, isOutput=True)

    sync_pages = list(range(F // 2))
    scalar_pages = list(range(F // 2, F))

    with (
        nc.sbuf_tensor([128, SUBS * 512], F32) as ptile,
        nc.sbuf_tensor([128, 512], F32) as htile,
        nc.semaphore("hsem") as hsem,
        nc.semaphore("psem") as psem,
        nc.semaphore("s_sem") as s_sem,
        nc.semaphore("a_sem") as a_sem,
        nc.Block() as block,
    ):
        @block.vector
        def _(vector):
            vector.memset(htile[:, :], float(h_fin)).then_inc(hsem, 1)
            for k in range(SUBS):
                l, line = divmod(k, 2)
                val = ks[l] if line == 0 else vs[l]
                vector.memset(ptile[:, k * 512:(k + 1) * 512], float(val)).then_inc(psem, 1)

        @block.sync
        def _(sync):
            n = 0
            for r in range(reps):
                for k in range(SUBS):
                    if r == 0:
                        sync.wait_ge(psem, k + 1)
                    for f in sync_pages:
                        sync.dma_start(
                            out=iop[f, k], in_=ptile[:, k * 512:(k + 1) * 512]
                        ).then_inc(s_sem, 16)
                        n += 16
            if n:
                sync.wait_ge(s_sem, n)

        @block.scalar
        def _(scalar):
            scalar.wait_ge(hsem, 1)
            n = 0
            for r in range(reps):
                for q in range(4):
                    scalar.dma_start(
                        out=outh[q * 128:(q + 1) * 128, :], in_=htile[:, :]
                    ).then_inc(a_sem, 16)
                    n += 16
                for k in range(SUBS):
                    if r == 0:
                        scalar.wait_ge(psem, k + 1)
                    for f in scalar_pages:
                        scalar.dma_start(
                            out=iop[f, k], in_=ptile[:, k * 512:(k + 1) * 512]
                        ).then_inc(a_sem, 16)
                        n += 16
            scalar.wait_ge(a_sem, n)

    _NC_CACHE[key] = nc
    return nc


def build_copy(F, C, reps=1):
    """Fallback without aliasing: copy pages DRAM->DRAM + fill pages from SBUF."""
    key = ("copy", F, C, reps)
    if key in _NC_CACHE:
        return _NC_CACHE[key]
    ks, vs, h_fin = _fill_values()
    nc = bass.Bass()
    src = nc.declare_dram_parameter("src_pages", [C, SUBS, 128, 512], F32, isOutput=False)
    outp = nc.declare_dram_parameter("out_pages", [F + C, SUBS, 128, 512], F32, isOutput=True)
    outh = nc.declare_dram_parameter("out_h", [512, 512], F32, isOutput=True)

    with (
        nc.sbuf_tensor([128, SUBS * 512], F32) as ptile,
        nc.sbuf_tensor([128, 512], F32) as htile,
        nc.semaphore("vsem") as vsem,
        nc.semaphore("s_sem") as s_sem,
        nc.semaphore("a_sem") as a_sem,
        nc.Block() as block,
    ):
        @block.vector
        def _(vector):
            vector.memset(htile[:, :], float(h_fin))
            for k in range(SUBS):
                l, line = divmod(k, 2)
                val = ks[l] if line == 0 else vs[l]
                vector.memset(ptile[:, k * 512:(k + 1) * 512], float(val))
            vector.sem_inc(vsem, 1)

        @block.sync
        def _(sync):
            n = 0
            for _ in range(reps):
                for j in range(C):
                    sync.dma_start(out=outp[F + j], in_=src[j]).then_inc(s_sem, 16)
                    n += 16
            sync.wait_ge(s_sem, n)

        @block.scalar
        def _(scalar):
            scalar.wait_ge(vsem, 1)
            n = 0
            for _ in range(reps):
                for q in range(4):
                    scalar.dma_start(
                        out=outh[q * 128:(q + 1) * 128, :], in_=htile[:, :]
                    ).then_inc(a_sem, 16)
                    n += 16
                for f in range(F):
                    for k in range(SUBS):
                        scalar.dma_start(
                            out=outp[f, k], in_=ptile[:, k * 512:(k + 1) * 512]
                        ).then_inc(a_sem, 16)
                        n += 16
            scalar.wait_ge(a_sem, n)

    _NC_CACHE[key] = nc
    return nc


def plan(attn_block_ids):
    """Host routing: fill pages (appear in attn_block_ids) and copy pages,
    assigned round-robin to cores, padded to uniform per-core counts."""
    ids = np.asarray(attn_block_ids).reshape(-1).astype(np.int64)
    ids = ids[(ids >= 0) & (ids < PAGES)]
    mask = np.zeros(PAGES, dtype=bool)
    mask[ids] = True
    fill_pages = np.nonzero(mask)[0]
    copy_pages = np.nonzero(~mask)[0]
    nf, ncp = len(fill_pages), len(copy_pages)
    F = max(2, -(-nf // N_CORES))
    C = max(1, -(-ncp // N_CORES))
    fill_asn = np.full(F * N_CORES, -1, dtype=np.int64)
    fill_asn[:nf] = fill_pages
    copy_asn = np.full(C * N_CORES, -1, dtype=np.int64)
    copy_asn[:ncp] = copy_pages
    return F, C, fill_asn.reshape(N_CORES, F), copy_asn.reshape(N_CORES, C)


def _routed_pages(slab_pages, F, C, fill_asn, copy_asn):
    """Per-core [G, SUBS, 128, 512] slot contents (dummy zeros for padding)."""
    per_core = []
    for c in range(N_CORES):
        sel = np.concatenate([fill_asn[c], copy_asn[c]])
        safe = np.where(sel >= 0, sel, 0)
        pages = slab_pages[safe].copy()
        pages[sel < 0] = 0.0
        per_core.append(pages)
    return per_core


def assemble(out_pages_per_core, out_h_per_core, F, fill_asn, copy_asn, slab_dtype):
    out_slab = np.empty((PAGES, SUBS, 128, 512), dtype=slab_dtype)
    for c in range(N_CORES):
        op = out_pages_per_core[c]
        for i, p in enumerate(fill_asn[c]):
            if p >= 0:
                out_slab[p] = op[i]
        for j, p in enumerate(copy_asn[c]):
            if p >= 0:
                out_slab[p] = op[F + j]
    h = np.concatenate([o.reshape(-1) for o in out_h_per_core]).reshape(BS, SEQ, FEAT)
    return h, out_slab.reshape(SLAB_SHAPE)


def _run_inplace_axon(nc, in_maps):
    """Custom exec path: bass2jax with out_pages aliased onto io_pages."""
    import jax
    import jax.numpy as jnp
    from jax.sharding import Mesh, PartitionSpec, NamedSharding
    from jax.experimental.shard_map import shard_map
    from concourse import bass2jax

    bass2jax.install_neuronx_cc_hook()
    devices = jax.devices()[:N_CORES]
    assert len(devices) == N_CORES
    mesh = Mesh(np.asarray(devices), ("core",))
    sh = NamedSharding(mesh, PartitionSpec("core"))

    partition_name = nc.partition_id_tensor.name if nc.partition_id_tensor else None
    in_names, out_names, out_avals, zero_shapes = [], [], [], []
    for alloc in nc.m.functions[0].allocations:
        if not isinstance(alloc, mybir.MemoryLocationSet):
            continue
        name = alloc.memorylocations[0].name
        if alloc.kind == "ExternalInput":
            if name != partition_name:
                in_names.append(name)
        elif alloc.kind == "ExternalOutput":
            out_names.append(name)
            shape = tuple(alloc.tensor_shape)
            dtype = mybir.dt.np(alloc.dtype)
            out_avals.append(jax.core.ShapedArray(shape, dtype))
            zero_shapes.append((shape, dtype))
    n_params = len(in_names)
    all_names = in_names + out_names
    if partition_name is not None:
        all_names = all_names + [partition_name]
    aliases = ((out_names.index("out_pages"), in_names.index("io_pages")),)

    def _body(*args):
        operands = list(args)
        if partition_name is not None:
            operands.append(bass2jax.partition_id_tensor())
        outs = bass2jax._bass_exec_p.bind(
            *operands,
            out_avals=tuple(out_avals),
            in_names=tuple(all_names),
            out_names=tuple(out_names),
            lowering_input_output_aliases=aliases,
            sim_require_finite=False,
            sim_require_nnan=False,
            nc=nc,
        )
        return tuple(outs)

    n_outs = len(out_names)
    fn = jax.jit(
        shard_map(_body, mesh=mesh,
                  in_specs=(PartitionSpec("core"),) * (n_params + n_outs),
                  out_specs=(PartitionSpec("core"),) * n_outs,
                  check_rep=False),
        donate_argnums=(0,) + tuple(range(n_params, n_params + n_outs)),
        keep_unused=True,
    )
    dins = [
        jax.device_put(np.concatenate([m[name] for m in in_maps], axis=0), sh)
        for name in in_names
    ]
    zouts = [
        jax.device_put(np.zeros((N_CORES * s[0],) + s[1:], d), sh)
        for s, d in zero_shapes
    ]
    outs = fn(*dins, *zouts)
    jax.block_until_ready(outs)
    results = []
    for c in range(N_CORES):
        res = {}
        for i, name in enumerate(out_names):
            g = np.asarray(outs[i])
            res[name] = g.reshape((N_CORES,) + out_avals[i].shape)[c]
        results.append(res)
    return results


def _spot_check(results, in_maps, F, C):
    """Verify aliasing passed copy pages through (not zeroed/garbage)."""
    for c in range(N_CORES):
        got = results[c]["out_pages"][F:F + C, 0, 0, :8]
        want = in_maps[c]["io_pages"][F:F + C, 0, 0, :8]
        if not np.array_equal(got, want):
            return False
    return True


def kernel(seq_lens=None, attn_block_ids=None, attn_page_slab=None, **_):
    slab = np.asarray(attn_page_slab)
    F, C, fill_asn, copy_asn = plan(attn_block_ids)
    slab_pages = slab.reshape(PAGES, SUBS, 128, 512)
    routed = _routed_pages(slab_pages, F, C, fill_asn, copy_asn)

    results = None
    try:
        nc = build_inplace(F, C)
        in_maps = [{"io_pages": routed[c]} for c in range(N_CORES)]
        if axon_active():
            results = _run_inplace_axon(nc, in_maps)
        else:
            results = run_bass_kernel_spmd(
                nc, in_maps, list(range(N_CORES)),
                aliases={"out_pages": "io_pages"},
            ).results
        if not _spot_check(results, in_maps, F, C):
            results = None
    except Exception:
        results = None

    if results is None:
        # Fallback: explicit-copy kernel, no aliasing required.
        nc = build_copy(F, C)
        in_maps = [{"src_pages": routed[c][F:]} for c in range(N_CORES)]
        results = run_bass_kernel_spmd(nc, in_maps, list(range(N_CORES))).results

    return assemble(
        [r["out_pages"] for r in results],
        [r["out_h"] for r in results],
        F, fill_asn, copy_asn, slab.dtype,
    )
